# revision 98
# baseline (speedup 1.0000x reference)
"""Trainium2 Bass kernel for nn_Attention_27376121544790.

Math (per batch element, B=8 -> one element per NeuronCore, no collectives):
  qk   = x @ W.T + b                              [N, D] (on device: [D, N])
  q = k = l2norm(qk per 64-dim head)
  S    = (q @ k.T) * (sqrt(64)/attn_gamma)        per head
  attn = softmax(S) = E / Z,  E = exp(S), Z = col sums (E symmetric)
  out  = attn @ v,  v = x head-split
  final= w0*(out @ W.T) + w1*qk + (1-w1)*b        (uses x@W.T = qk - b, so the
         blend projection collapses into the already-computed qk)

Engine plan (exp of the N^2 logits dominates; the design splits it across
ACT and DVE and keeps every serial chain short):
  - attention-path proj1 (qk for the norm/gram) is all-fp8 DoubleRow off
    W8 (shared with proj2) + x8: only 2MB of DMA gates the startup. The
    accurate bf16 proj1 (wx) runs later, feeding only the blend addend
    ad = (w1*qk + (1-w1)*b)/W0S (fused scale+bias drain into the dead qk
    tiles). Startup qk drains for chunks 1-2 go to the still-idle ACT.
  - exp stream: 5-of-6 logit blocks on ACT (Exp -> fp8); every 6th block
    on DVE as a ONE-OP Schraudolph straight into fp8e4m3 bits
    (bits = A*logit + B, log-space rounding == fp8 mantissa rounding).
  - l2norm in TRANSPOSED [128,16] layout: ssqT via sq-stationary matmuls
    (out free = 2), quake rsqrt chain at free-16 cost (x8 fp8 scale baked
    into the seed constant), PE-transpose back, selector-matmul broadcast
    into pbt psum. Chunk 0's sq comes from idle-ACT Square directly off
    the proj1 psum.
  - Z: pav carries a 1/32-ones column; 1/Z via native DVE InstReciprocal
    READING PSUM DIRECTLY (no staging copy), split per fn-half so att
    columns stream early; partition broadcast on gpsimd.
  - final projection: psum = att@W8 (+ ad via identity matmul on odd m),
    drains alternate DVE tensor-add / ACT copy; the global W0S scale is
    folded into the host-side unshard.
  - PE p-state warmup before the first matmul; a short filler block keeps
    the PE clocked through the last head's Z chain.
  - pipeline: fp8 proj1 two windows ahead, norm one ahead (emitted before
    the chunk's first head so the qn8 relayout DMAs round-trip early).
"""

import math
import os

import numpy as np

B, N, C, D = 8, 1024, 1024, 1024
HEADS, HD = 16, 64
P = 128
EPS = 1e-6
NCHUNK = C // P      # 8 chunks of 128 feature rows
NPAIR = NCHUNK // 2  # 4 DoubleRow pair chunks
FH = 512             # free-dim half (one PSUM bank of f32)
HP = 80              # padded per-head stride in xaug (65 used, %16 == 0)
W8S = 32.0           # fp8 scale on W
VSC = 32.0           # att scale (via 1/32 ones column)


def _build(gamma: float, w0: float, w1: float, logit_scale: float):
    import concourse.bass as bass
    import concourse.tile as tile
    from concourse import bacc, mybir

    f32 = mybir.dt.float32
    f32r = mybir.dt.float32r
    BF16 = mybir.dt.bfloat16
    FP8 = mybir.dt.float8e4
    DR = mybir.MatmulPerfMode.DoubleRow

    Exp = mybir.ActivationFunctionType.Exp
    MULT = mybir.AluOpType.mult
    ADD = mybir.AluOpType.add
    SUB = mybir.AluOpType.subtract
    LSR = mybir.AluOpType.logical_shift_right
    i32 = mybir.dt.int32
    i8 = mybir.dt.int8
    QC = 0x5F3759DF  # quake rsqrt seed constant
    # Schraudolph exp straight into fp8e4m3 bits: bits = A*logit + B.
    # Rounding in log space is +-0.5/8 octave = +-4.4%, the same magnitude
    # as rounding true exp() to fp8's 3-bit mantissa. B carries the
    # standard -0.0430-octave bias to centre the (1+f) vs 2^f sawtooth.
    EXP8_SHIFT = 0.5  # convert-rounds-toward: +0.5 if convert truncates

    W0S = w0 / (W8S * VSC)  # proj2 drain scale

    nc = bacc.Bacc("TRN2", target_bir_lowering=False, debug=False)

    wx_d = nc.declare_dram_parameter("wx", [C, 2, N], BF16, isOutput=False)
    x8_d = nc.declare_dram_parameter("x8", [NPAIR * P, 2, N], FP8, isOutput=False)
    W8_d = nc.declare_dram_parameter("W8", [NPAIR * P, 2, D], FP8, isOutput=False)
    xa_d = nc.declare_dram_parameter("xa", [NPAIR * P, 2, HEADS * HP], FP8, isOutput=False)
    bdc_d = nc.declare_dram_parameter("bdc", [P, 2], BF16, isOutput=False)
    sel_d = nc.declare_dram_parameter("selm", [16, NCHUNK * P], BF16, isOutput=False)
    id_d = nc.declare_dram_parameter("idn", [P, P], f32r, isOutput=False)
    bmat_d = nc.declare_dram_parameter("bmat", [P, NCHUNK], f32, isOutput=False)
    bmat2_d = nc.declare_dram_parameter("bmat2", [P, NCHUNK], f32, isOutput=False)
    out_d = nc.declare_dram_parameter("out", [D, N], BF16, isOutput=True)

    with tile.TileContext(nc) as tc:
        with (
            tc.tile_pool(name="pers", bufs=1) as pers,
            tc.tile_pool(name="small", bufs=1) as small,
            tc.tile_pool(name="sqp", bufs=int(os.environ.get("BK_SQP", "2"))) as sqp,
            tc.tile_pool(name="invp", bufs=int(os.environ.get("BK_INVP", "2"))) as invp,
            tc.tile_pool(name="qnp", bufs=int(os.environ.get("BK_QNP", "2"))) as qnp,
            tc.tile_pool(name="fin", bufs=int(os.environ.get("BK_FIN", "8"))) as pfin,
            tc.tile_pool(name="psum_p1", bufs=2, space="PSUM") as pp1,
        ):
            # ---- persistent SBUF ----
            wx_t = [pers.tile([P, 2, N], BF16, tag=f"wx{c}", name=f"wx{c}") for c in range(NCHUNK)]
            x8_t = [pers.tile([P, 2, N], FP8, tag=f"x8{p}", name=f"x8{p}") for p in range(NPAIR)]
            W8_t = [pers.tile([P, 2, D], FP8, tag=f"W8{p}", name=f"W8{p}") for p in range(NPAIR)]
            xa_t = [pers.tile([P, 2, HEADS * HP], FP8, tag=f"xa{p}", name=f"xa{p}") for p in range(NPAIR)]
            qkT_t = [pers.tile([P, N], f32r, tag=f"qk{c}", name=f"qk{c}") for c in range(NCHUNK)]
            qn8_t = [pers.tile([32, 2, 2, N], FP8, tag=f"q8{c}", name=f"q8{c}") for c in range(NCHUNK)]
            att_t = [pers.tile([P, 2, N], FP8, tag=f"at{p}", name=f"at{p}") for p in range(NPAIR)]
            # ad aliases qk: chunk c's qk tile is dead once sq/qn consumed it,
            # and the tile framework orders the ad write behind those reads
            ad_t = qkT_t

            bdc_t = small.tile([P, 2], BF16, tag="bdc")
            sel_t = small.tile([16, NCHUNK * P], BF16, tag="selm")
            bmat_t = small.tile([P, NCHUNK], f32, tag="bmat")
            bmat2_t = small.tile([P, NCHUNK], f32, tag="bmat2")
            id_t = small.tile([P, P], f32r, tag="idn")

            # PE p-state warmup: ~5us of dependency-free matmuls during the
            # input-DMA wait ramps the tensor engine to full clock; the ramp
            # state persists across later idle gaps.
            warm_t = small.tile([P, FH], BF16, tag="warm")
            nc.gpsimd.memset(warm_t[:], 0.25)
            wps = pp1.tile([P, FH], f32, tag="p1", name="wps")
            for i in range(int(os.environ.get("BK_WARM", "8"))):
                nc.tensor.matmul(wps[:], warm_t[:, 0:P], warm_t[:],
                                 start=True, stop=True)

            # DMA order drives the startup critical path: bias first, then
            # the fp8 proj1 operands (W8|x8 interleaved so the contraction
            # chases arrivals), then the small norm constants, then xa (first
            # pav), and only then the big bf16 wx (addend path, lots of
            # slack).
            nc.sync.dma_start(bmat_t[:], bmat_d[:])
            for p in range(NPAIR):
                nc.sync.dma_start(W8_t[p][:], W8_d[p * P:(p + 1) * P, :, :])
                nc.sync.dma_start(x8_t[p][:], x8_d[p * P:(p + 1) * P, :, :])
            nc.sync.dma_start(bdc_t[:], bdc_d[:])
            nc.sync.dma_start(sel_t[:], sel_d[:])
            nc.sync.dma_start(id_t[:], id_d[:])
            nc.sync.dma_start(bmat2_t[:], bmat2_d[:])
            for p in range(NPAIR):
                nc.sync.dma_start(xa_t[p][:], xa_d[p * P:(p + 1) * P, :, :])
            for c in range(NCHUNK):
                nc.sync.dma_start(wx_t[c][:], wx_d[c * P:(c + 1) * P, :, :])

            Ident = mybir.ActivationFunctionType.Identity
            p1ps0 = []  # chunk-0 proj1 psum tiles (reused by the ACT sq)

            def emit_proj1(c):
                # attention-path qk_c = (xp @ W.T)_c + b_c, all-fp8 DoubleRow
                # (W8 doubles as the proj2 weight; psum carries 32*qk).
                # Chunks 1-2 drain on ACT (idle before the exp stream) so the
                # startup norm chain runs back-to-back on DVE.
                for fn in range(2):
                    ps = pp1.tile([P, FH], f32, tag="p1", name="ps")
                    if c == 0:
                        p1ps0.append(ps)
                    for kp in range(NPAIR):
                        nc.tensor.matmul(
                            ps[:],
                            W8_t[kp][:, :, c * P:(c + 1) * P],
                            x8_t[kp][:, :, fn * FH:(fn + 1) * FH],
                            start=(kp == 0), stop=(kp == NPAIR - 1),
                            perf_mode=DR)
                    if c in (1, 2):
                        nc.scalar.activation(
                            qkT_t[c][:, fn * FH:(fn + 1) * FH], ps[:],
                            Ident, bias=bmat_t[:, c:c + 1], scale=1.0 / W8S)
                    else:
                        nc.vector.tensor_scalar(
                            qkT_t[c][:, fn * FH:(fn + 1) * FH], ps[:],
                            1.0 / W8S, bmat_t[:, c:c + 1], MULT, ADD)

            def emit_proj1_ad(c, fns=(0, 1)):
                # accurate bf16 x @ W.T for the final-output addend:
                # ad_c = (w1*qk_c + (1-w1)*b_c)/W0S, drained fused
                for fn in fns:
                    ps = pp1.tile([P, FH], f32, tag="p1", name="ps0")
                    for k in range(NCHUNK):
                        nc.tensor.matmul(
                            ps[:],
                            wx_t[k][:, 0, c * P:(c + 1) * P],
                            wx_t[k][:, 1, fn * FH:(fn + 1) * FH],
                            start=(k == 0), stop=(k == NCHUNK - 1))
                    nc.vector.tensor_scalar(
                        ad_t[c][:, fn * FH:(fn + 1) * FH], ps[:],
                        float(w1 / W0S), bmat2_t[:, c:c + 1], MULT, ADD)

            qn_stage = {}

            def emit_norm(c, psum_pool=None):
                # l2norm pair for heads (2c, 2c+1), all in TRANSPOSED
                # [128-token-part, 16] layout so every elementwise op costs
                # ~free-size-16 instead of free-size-1024:
                #   ssqT[n, (j,s)] via sq-stationary matmuls (out free = 2),
                #   quake rsqrt chain on [128, 16] (the x8 fp8 scale rides in
                #   the magic constant), PE-transpose back to rows, selector
                #   matmuls broadcast into pbt psum.
                import contextlib
                prio = tc.high_priority() if c == 0 else contextlib.nullcontext()
                with prio:
                    return emit_norm_body(c, psum_pool)

            def emit_norm_body(c, psum_pool=None):
                pool = psum_pool if psum_pool is not None else pp1
                ptag = "p1" if psum_pool is None else "pg"
                sq = sqp.tile([P, N], BF16, tag="sq", name="sq")
                if c == 0:
                    # startup: sq on the idle ACT straight from the proj1
                    # psum (Square(ps/32 + b)), in parallel with the DVE
                    # qk drain -- takes sq off the serial startup chain
                    Sqr = mybir.ActivationFunctionType.Square
                    for fn, pst in enumerate(p1ps0):
                        nc.scalar.activation(
                            sq[:, fn * FH:(fn + 1) * FH], pst[:],
                            Sqr, bias=bmat_t[:, 0:1], scale=1.0 / W8S)
                else:
                    # two halves so a head's Z-broadcast can slot between
                    # them in the in-order Pool queue
                    for fn in range(2):
                        nc.gpsimd.tensor_mul(
                            sq[:, fn * FH:(fn + 1) * FH],
                            qkT_t[c][:, fn * FH:(fn + 1) * FH],
                            qkT_t[c][:, fn * FH:(fn + 1) * FH])
                ssqT = pool.tile([P, 16], f32, tag=ptag, name="ssqT")
                for j in range(NCHUNK):
                    nc.tensor.matmul(
                        ssqT[:, 2 * j:2 * j + 2],
                        sq[:, j * P:(j + 1) * P], bdc_t[:],
                        start=True, stop=True)
                sdT = invp.tile([P, 16], f32, tag="sd", name="sdT")
                nc.vector.tensor_copy(sdT[:], ssqT[:])
                y0 = invp.tile([P, 16], i32, tag="y0", name="y0")
                t2 = invp.tile([P, 16], f32, tag="t2", name="t2")
                inv = invp.tile([P, 16], f32r, tag="inv", name="inv")
                # quake chain on [128, 16] (x8 fp8 scale in the seed const)
                nc.vector.tensor_scalar(y0[:], sdT[:].bitcast(i32), 1,
                                        None, LSR)
                y0f = y0[:].bitcast(f32)
                nc.vector.tensor_scalar(y0[:], y0[:], QC + (3 << 23), -1,
                                        SUB, MULT)
                nc.vector.tensor_mul(t2[:], y0f, y0f)
                nc.vector.tensor_mul(sdT[:], sdT[:], t2[:])
                # Newton (with y0 8x-scaled): inv8 = y0*(1.5 - x*y0^2/128)
                nc.vector.tensor_scalar(t2[:], sdT[:], -1.0 / 128.0, 1.5,
                                        MULT, ADD)
                nc.vector.tensor_mul(inv[:], t2[:], y0f)
                # transpose [128, 16] -> [16, 128] and stage as bf16 rows
                invT = pool.tile([16, P], f32, tag=ptag, name="invT")
                nc.tensor.transpose(invT[:].bitcast(f32r), inv[:], id_t[:])
                invr = invp.tile([16, P], BF16, tag="invr", name="invr")
                nc.vector.tensor_copy(invr[:], invT[:])

                # qn = qk * bcast(8*invn) -> fp8; bcast via selector matmuls
                qn = qnp.tile([P, N], FP8, tag="qn", name="qn")
                for fn in range(2):
                    pbt = pool.tile([P, FH], f32, tag=ptag, name="pbt")
                    for jj in range(4):
                        j = 4 * fn + jj
                        nc.tensor.matmul(
                            pbt[:, jj * P:(jj + 1) * P],
                            sel_t[:, j * P:(j + 1) * P], invr[:],
                            start=True, stop=True)
                    nc.vector.tensor_mul(
                        qn[:, fn * FH:(fn + 1) * FH],
                        qkT_t[c][:, fn * FH:(fn + 1) * FH], pbt[:, 0:FH])
                    # relayout into DoubleRow [32, 2, *] per (half, fn) so
                    # each piece DMAs as soon as its qn column half lands
                    # (flat AP pairing maps head-dim d = 2p+s, valid because
                    # the gram uses the same tile on both sides)
                    if c != 0:
                        for half in range(2):
                            nc.sync.dma_start(
                                qn8_t[c][:, half, :,
                                         fn * FH:(fn + 1) * FH],
                                qn[half * HD:(half + 1) * HD,
                                   fn * FH:(fn + 1) * FH])
                qn_stage[c] = qn

            with (
                tc.tile_pool(name="E", bufs=int(os.environ.get("BK_EBUF", "7"))) as pE,
                tc.tile_pool(name="rzp", bufs=int(os.environ.get("BK_RZP", "3"))) as rzp,
                tc.tile_pool(name="psum_g", bufs=2, space="PSUM") as pg_pool,
                tc.tile_pool(name="psum_av", bufs=1, space="PSUM") as pav_pool,
            ):
                last_E = []

                # Schraudolph constants: E-fp8-bits = EA*pg + EB on DVE
                EA = 8.0 * math.log2(math.e) * logit_scale / 64.0
                EB = 56.0 - 0.344 + EXP8_SHIFT
                EXPR = int(os.environ.get("BK_EXPR", "6"))

                def emit_head(h):
                    c, half = h // 2, h % 2
                    pav = pav_pool.tile([HD + 1, 2, FH], f32, tag="pav", name="pav")
                    for p in range(NPAIR):
                        Ep = pE.tile([P, 2, N], FP8, tag="E", name="Ep")
                        if h == 2 * NCHUNK - 1:
                            last_E.append(Ep)
                        for s in range(2):
                            mb = 2 * p + s
                            pg = pg_pool.tile([P, N], f32, tag="pg", name="pg")
                            for fn in range(2):
                                if c == 0:
                                    # chunk 0: plain fp8 matmul off the qn
                                    # staging tile (no relayout wait)
                                    qs = qn_stage[0][half * HD:(half + 1) * HD, :]
                                    nc.tensor.matmul(
                                        pg[:, fn * FH:(fn + 1) * FH],
                                        qs[:, mb * P:(mb + 1) * P],
                                        qs[:, fn * FH:(fn + 1) * FH],
                                        start=True, stop=True)
                                else:
                                    qn_h = qn8_t[c][:, half, :, :]
                                    nc.tensor.matmul(
                                        pg[:, fn * FH:(fn + 1) * FH],
                                        qn_h[:, :, mb * P:(mb + 1) * P],
                                        qn_h[:, :, fn * FH:(fn + 1) * FH],
                                        start=True, stop=True, perf_mode=DR)
                            v = 2 * p + s
                            pat = int(os.environ.get("BK_EPAT", "0"))
                            if pat == 3:
                                use_dve = (v in (2, 5)) if h % 2 == 0 else (v == 3)
                            elif pat == 4:
                                use_dve = v in (2, 5) and h % 4 != 3
                            else:
                                use_dve = (v + h * int(os.environ.get("BK_HSTEP", "3"))) % EXPR == 0
                            if os.environ.get("BK_LASTV7", "1") == "1":
                                # run the very last logit block on DVE, in
                                # parallel with ACT's second-to-last: the
                                # tail starts when this block lands
                                use_dve = use_dve or (
                                    h >= 2 * NCHUNK - 1 - int(
                                        os.environ.get("BK_LASTN", "3"))
                                    and v == 7)
                            if EXPR > 0 and use_dve:
                                # offload 1/EXPR of the exp stream to DVE:
                                # Schraudolph straight into fp8 bits
                                nc.vector.tensor_scalar(
                                    Ep[:, s, :].bitcast(i8), pg[:],
                                    EA, EB, MULT, ADD)
                            else:
                                nc.scalar.activation(Ep[:, s, :], pg[:], Exp,
                                                     scale=logit_scale / 64.0)
                        for fn in range(2):
                            nc.tensor.matmul(
                                pav[:, fn, :],
                                xa_t[p][:, :, h * HP:h * HP + HD + 1],
                                Ep[:, :, fn * FH:(fn + 1) * FH],
                                start=(p == 0), stop=(p == NPAIR - 1),
                                perf_mode=DR)
                    # InstReciprocal (native DVE, IEEE 1/x) reads PSUM
                    # directly -- no SBUF staging copy. The whole Z chain is
                    # split per fn-half so att columns stream out ~1.5us
                    # earlier (matters for the last head's tail).
                    rz = rzp.tile([1, 2, FH], f32, tag="rz", name="rz")
                    rzb = rzp.tile([P, N], f32, tag="rzb", name="rzb")
                    ap, sp = h // 4, (h // 2) % 2
                    # both recips FIRST (the DVE queue is in-order: attmul-fn0
                    # must not block recip-fn1), then broadcasts, then muls
                    for fn in range(2):
                        nc.vector.reciprocal(
                            rz[:, fn, :], pav[HD:HD + 1, fn, :])
                        nc.gpsimd.partition_broadcast(
                            rzb[:, fn * FH:(fn + 1) * FH], rz[:, fn, :])
                    for fn in range(2):
                        nc.vector.tensor_mul(
                            att_t[ap][half * HD:(half + 1) * HD, sp,
                                      fn * FH:(fn + 1) * FH],
                            pav[0:HD, fn, :],
                            rzb[half * HD:(half + 1) * HD,
                                fn * FH:(fn + 1) * FH])

                # software pipeline: fp8 proj1 two windows ahead (chunks 1-2
                # drain on the still-idle ACT so the startup norm chain owns
                # the DVE queue), norm one ahead; the bf16 addend projection
                # rides a window behind (slack until the final projection).
                emit_proj1(0)
                emit_norm(0, pg_pool)
                emit_proj1(1)
                for c in range(NCHUNK):
                    if c + 2 < NCHUNK:
                        emit_proj1(c + 2)
                    if c + 1 < NCHUNK:
                        emit_norm(c + 1)
                    emit_head(2 * c)
                    if c >= 1:
                        emit_proj1_ad(c - 1)
                    emit_head(2 * c + 1)
                emit_proj1_ad(NCHUNK - 1)

                # keep the tensor engine clocked through the last head's
                # serial Z-normalization chain: these fillers depend on the
                # final E tile, so the in-order PE stream runs them in the
                # otherwise-idle gap right before the final projection
                wps2 = pp1.tile([P, FH], f32, tag="p1", name="wps2")
                for i in range(int(os.environ.get("BK_FILL", "8"))):
                    nc.tensor.matmul(wps2[:], last_E[NPAIR - 1][:, 1, 0:P],
                                     warm_t[:], start=True, stop=True)

                if os.environ.get("BK_DEBUG", "0") == "1":
                    dbg = {
                        "d_qk0": qkT_t[0],
                        "d_qn8_1": qn8_t[1],
                        "d_att0": att_t[0],
                        "d_xa0": xa_t[0],
                        "d_w80": W8_t[0],
                    }
                    for nm, t in dbg.items():
                        sh = list(t[:].shape)
                        flat = [sh[0], int(np.prod(sh[1:]))]
                        dd = nc.declare_dram_parameter(nm, flat, t.tensor.dtype,
                                                       isOutput=True)
                        nc.sync.dma_start(dd[:], t[:])

                # ---- final projection (W0S folded into the host unshard):
                # odd m: psum = att@W8 + ad via identity matmul, drain = pure
                #   ACT copies;
                # even m: psum = att@W8, drain = DVE tensor-add with ad.
                # The two drain flavours run on different engines, so the 8
                # tail drains pipeline two-wide.
                m_order = (list(range(NCHUNK))
                           if os.environ.get("BK_MORD", "0") == "0" else
                           [1, 0, 3, 2, 5, 4, 7, 6])
                for m in m_order:
                    fin = pfin.tile([P, N], BF16, tag="fin", name="fin")
                    if m % 2 == 0:
                        ps2f = [pg_pool.tile([P, N], f32, tag="pg", name="ps2")]
                        slc = [(ps2f[0], fn * FH) for fn in range(2)]
                    else:
                        a = pp1.tile([P, FH], f32, tag="p1", name="p2a")
                        bb = pp1.tile([P, FH], f32, tag="p1", name="p2b")
                        slc = [(a, 0), (bb, 0)]
                    for fn in range(2):
                        t, off = slc[fn]
                        if m % 2 == 1:
                            nc.tensor.matmul(
                                t[:, off:off + FH],
                                id_t[:],
                                ad_t[m][:, fn * FH:(fn + 1) * FH],
                                start=True, stop=False)
                        for p in range(NPAIR):
                            nc.tensor.matmul(
                                t[:, off:off + FH],
                                W8_t[p][:, :, m * P:(m + 1) * P],
                                att_t[p][:, :, fn * FH:(fn + 1) * FH],
                                start=(m % 2 == 0 and p == 0),
                                stop=(p == NPAIR - 1),
                                perf_mode=DR)
                        if m % 2 == 1:
                            nc.scalar.copy(
                                fin[:, fn * FH:(fn + 1) * FH], t[:, 0:FH])
                    if m % 2 == 0:
                        nc.vector.tensor_add(fin[:], slc[0][0][:], ad_t[m][:])
                    nc.sync.dma_start(out_d[m * P:(m + 1) * P, :], fin[:])

    nc.compile()
    return nc


def _host_prep(x, pos, W, b, gamma, w0, w1):
    """Per-core input shards (host layout work only)."""
    import ml_dtypes

    bf16 = ml_dtypes.bfloat16
    f8 = ml_dtypes.float8_e4m3

    WT = np.ascontiguousarray(W.T)                        # [C, D] f32
    WTb = WT.astype(bf16)                                 # [C, D] bf16
    # pair layout: W8[pair*128 + part, s, d] = 32*W.T[128*(2*pair+s)+part, d]
    W8 = (W8S * WT).astype(f8).reshape(NPAIR, 2, P, D).transpose(0, 2, 1, 3)
    W8 = np.ascontiguousarray(W8).reshape(NPAIR * P, 2, D)
    bmat = np.ascontiguousarray(b.reshape(NCHUNK, P).T)   # [P, 8]
    w0s = w0 / (W8S * VSC)
    bmat2 = np.ascontiguousarray((b / w0s).reshape(NCHUNK, P).T)
    idn = np.eye(P, dtype=np.float32)
    bdc = np.zeros((P, 2), dtype=bf16)
    bdc[:HD, 0] = 1.0
    bdc[HD:, 1] = 1.0
    # selector for the inv-row broadcast matmuls: selm[2j+s, j*P+d] = 1 where
    # s == d>=64 (the 8x fp8 scale rides in the quake seed constant)
    selm = np.zeros((16, NCHUNK * P), dtype=bf16)
    for j in range(NCHUNK):
        selm[2 * j, j * P:j * P + HD] = 1.0
        selm[2 * j + 1, j * P + HD:(j + 1) * P] = 1.0

    in_maps = []
    for i in range(B):
        xi = x[i]                                         # [N, C]
        xa = np.zeros((N, HEADS, HP), dtype=np.float32)
        xa[:, :, :HD] = xi.reshape(N, HEADS, HD)
        xa[:, :, HD] = 1.0 / VSC
        xa8 = xa.astype(f8).reshape(NPAIR, 2, P, HEADS * HP)
        xa8 = np.ascontiguousarray(xa8.transpose(0, 2, 1, 3))
        m = {
            "W8": W8,
            "xa": xa8.reshape(NPAIR * P, 2, HEADS * HP),
            "bdc": bdc,
            "selm": selm,
            "idn": idn,
            "bmat": bmat,
            "bmat2": bmat2,
        }
        # wx carries W.T and PLAIN x.T (bf16): the accurate addend path.
        # x8 carries the (pos-shifted) attention-path x in fp8 pair layout.
        wx = np.empty((C, 2, N), dtype=bf16)
        wx[:, 0, :] = WTb
        wx[:, 1, :] = xi.T.astype(bf16)
        xp = xi if gamma == 0.0 else xi + gamma * pos[i].reshape(C, N).T
        x8 = xp.T.astype(f8).reshape(NPAIR, 2, P, N).transpose(0, 2, 1, 3)
        m["x8"] = np.ascontiguousarray(x8).reshape(NPAIR * P, 2, N)
        m["wx"] = wx
        in_maps.append(m)
    return in_maps


LAST_RESULT = None


def kernel(x, pos, W, b, gamma, attn_gamma, sum_gamma0, sum_gamma1):
    global LAST_RESULT
    import sys
    sys.path.insert(0, "/opt/trn_rl_repo")
    from concourse.bass_utils import run_bass_kernel_spmd

    x = np.asarray(x, dtype=np.float32)
    pos = np.asarray(pos, dtype=np.float32)
    W = np.asarray(W, dtype=np.float32)
    b = np.asarray(b, dtype=np.float32)
    gamma = float(np.asarray(gamma))
    attn_gamma = float(np.asarray(attn_gamma))
    g0 = math.exp(float(np.asarray(sum_gamma0)))
    g1 = math.exp(float(np.asarray(sum_gamma1)))
    w0, w1 = g0 / (g0 + g1), g1 / (g0 + g1)
    logit_scale = math.sqrt(HD) / attn_gamma

    nc = _build(gamma, w0, w1, logit_scale)
    in_maps = _host_prep(x, pos, W, b, gamma, w0, w1)
    res = run_bass_kernel_spmd(
        nc, in_maps, core_ids=list(range(B)),
        trace=os.environ.get("BK_TRACE", "0") == "1",
    )
    LAST_RESULT = res
    # the device stores fin/W0S (pure att@W8 + prescaled addend); apply the
    # global drain scale here on the host
    w0s = w0 / (W8S * VSC)
    out = np.empty((B, N, D), dtype=np.float32)
    for i in range(B):
        out[i] = res.results[i]["out"].astype(np.float32).T * w0s
    return out

